# revision 29
# baseline (speedup 1.0000x reference)
"""Trainium2 Bass kernel for gated multi-head attention + residual + LayerNorm.

Problem (nn_CNP_5669356834854):
    B=2, L=2048, D=1024, H=16, DK=DV=64
    Q = q@wq.T+bq; K = k@wk.T+bk; V = v@wv.T+bv   (per-head split)
    attn = softmax((Q K^T / sqrt(DK)) * k_gate  [masked])
    out = LayerNorm(attn @ V @ wo.T + bo + q)

Sharding: 8 cores = (batch b in {0,1}) x (head-group hg in {0..3}, 4 heads each).
Launch 1 computes normalized per-head attention outputs O^T per core.
Launch 2 shards (batch, 512-row chunk) for the output projection + residual + LN.

Everything is computed in "T-space" (transposed layouts) so that no on-chip
transposes are needed:
    S^T[lk,lq] = matmul with lhsT=K^T tile, rhs=Q^T
    P^T = exp(S^T * gate^T - 20)        (the -20 cancels in normalization)
    O_aug = [V | ones64]^T-matmul: rows 0:64 = unnormalized O^T, rows 64:128 =
            the softmax denominator replicated across 64 partitions (free
            broadcast), so normalization is one reciprocal + one multiply.

Pipeline notes:
  - S^T PSUM tiles are [128, 1024] (one 512-lq chunk x 2 heads), double
    buffered, so the S matmul of the next chunk overlaps the gate-multiply of
    the current one and the PE stays continuously busy (p-state ramp).
  - The gate multiply is split between the Vector engine and the otherwise
    idle GpSimd engine (every 3rd tile).
  - The gate is host-repacked into the exact consumption layout (one DMA per
    (head-pair, lq-half, lk-tile)) and stored fp8-e4m3 to halve HBM traffic.
  - PV matmuls are emitted one lk-tile behind the S matmuls so the PE never
    waits on the mul+exp chain.
"""

import os

import numpy as np
import ml_dtypes

import concourse.bacc as bacc
import concourse.tile as tile
from concourse import mybir
from concourse.bass_utils import run_bass_kernel_spmd

B, L, D, H, DK, DV = 2, 2048, 1024, 16, 64, 64
EPS = 1e-5
NCORE = 8
HPC = 4  # heads per core
NKC = D // 128  # 8 contraction chunks
NLKT = L // 128  # 16 lk tiles
NCH = 4  # lq chunks
CH = L // NCH  # 512
MPC = HPC * DK  # 256 projected rows per core
EXP_BIAS = -20.0
HF = L // 2

GATE_FP8 = os.environ.get("GATE_FP8", "1") == "1"
GP_MOD = int(os.environ.get("GP_MOD", "0"))  # every GP_MOD'th mul on gpsimd (0=off; gpsimd cannot read PSUM)

F32 = mybir.dt.float32
BF16 = mybir.dt.bfloat16
FP8 = mybir.dt.float8e4
NPBF16 = ml_dtypes.bfloat16
NPFP8 = mybir.dt.np(FP8)
AF = mybir.ActivationFunctionType
AOP = mybir.AluOpType


def _bf(x):
    return np.ascontiguousarray(x).astype(NPBF16)


def _kc_layout(a):
    """[D, N] -> [128, NKC, N] with row r = kc*128+p  ->  [p, kc, :]."""
    d, n = a.shape
    assert d == NKC * 128
    return np.ascontiguousarray(a.reshape(NKC, 128, n).transpose(1, 0, 2))


def build_l1(masked: bool, use_bq: bool, use_bk: bool, use_bv: bool,
             gate_fp8: bool, gp_mod: int):
    nc = bacc.Bacc("TRN2", target_bir_lowering=False)
    gdt = FP8 if gate_fp8 else BF16

    qT = nc.declare_dram_parameter("qT", [128, NKC, L], BF16, isOutput=False)
    kT = nc.declare_dram_parameter("kT", [128, NKC, L], BF16, isOutput=False)
    vT = nc.declare_dram_parameter("vT", [128, NKC, L], BF16, isOutput=False)
    wqT = nc.declare_dram_parameter("wqT", [128, NKC, MPC], BF16, isOutput=False)
    wkT = nc.declare_dram_parameter("wkT", [128, NKC, MPC], BF16, isOutput=False)
    wvT = nc.declare_dram_parameter("wvT", [128, NKC, MPC], BF16, isOutput=False)
    # gate, host-packed in consumption order: block i = ((pr*2+half)*NLKT+lkt),
    # cols c*1024 + hp*512 + j
    gP = nc.declare_dram_parameter("gP", [4 * NLKT, 128, L], gdt, isOutput=False)
    if use_bq:
        bqP = nc.declare_dram_parameter("bqP", [128, 2], F32, isOutput=False)
    if use_bk:
        bkP = nc.declare_dram_parameter("bkP", [128, 2], F32, isOutput=False)
    if use_bv:
        bvR = nc.declare_dram_parameter("bvR", [1, MPC], F32, isOutput=False)
    if masked:
        mbT = nc.declare_dram_parameter("mbT", [L, L], BF16, isOutput=False)
    oT = nc.declare_dram_parameter("oT", [128, 2, L], BF16, isOutput=True)

    with tile.TileContext(nc) as tc:
        with (
            tc.tile_pool(name="xs", bufs=2) as xs,
            tc.tile_pool(name="ws", bufs=1) as ws,
            tc.tile_pool(name="qk", bufs=1) as qk,
            tc.tile_pool(name="gp", bufs=6) as gp,
            tc.tile_pool(name="tp", bufs=3) as tp,
            tc.tile_pool(name="pp", bufs=7) as pp,
            tc.tile_pool(name="op", bufs=1) as opl,
            tc.tile_pool(name="rp", bufs=2) as rp,
            tc.tile_pool(name="ps_s", bufs=2, space="PSUM") as ps_s,
            tc.tile_pool(name="ps_o", bufs=2, space="PSUM") as ps_o,
        ):
            wv_sb = ws.tile([128, NKC, MPC], BF16, tag="wv")
            nc.sync.dma_start(out=wv_sb, in_=wvT[:, :, :])
            wk_sb = ws.tile([128, NKC, MPC], BF16, tag="wk")
            nc.sync.dma_start(out=wk_sb, in_=wkT[:, :, :])
            wq_sb = ws.tile([128, NKC, MPC], BF16, tag="wq")
            nc.sync.dma_start(out=wq_sb, in_=wqT[:, :, :])

            QT = qk.tile([128, 2, L], BF16, tag="qt")
            KT = qk.tile([128, 2, L], BF16, tag="kt")
            Vaug = qk.tile([128, NLKT, HPC, 128], BF16, tag="va")
            warm = ws.tile([128, 2, 128], BF16, tag="warm")
            nc.vector.memset(warm, 1.0)
            nc.vector.memset(Vaug[:, :, :, 64:128], 1.0)
            ebias = ws.tile([128, 1], F32, tag="eb")
            nc.vector.memset(ebias, EXP_BIAS)

            # dummy matmuls while the first DMAs land: warms the ldweights
            # path and holds the PE p-state at full clock so the real
            # projections start fast
            junk = ps_s.tile([128, 1024], F32, tag="s", name="junk")
            for _ in range(80):
                nc.tensor.matmul(
                    junk[:, 0:128],
                    lhsT=warm[:, 0, :],
                    rhs=warm[:, 1, :],
                    start=True,
                    stop=True,
                )
            nc.scalar.copy(out=warm[0:1, 0, :], in_=junk[0:1, 0:128])

            bias_tiles = {}
            if use_bq:
                bq_sb = ws.tile([128, 2], F32, tag="bq")
                nc.sync.dma_start(out=bq_sb, in_=bqP[:, :])
                bias_tiles["q"] = bq_sb
            if use_bk:
                bk_sb = ws.tile([128, 2], F32, tag="bk")
                nc.sync.dma_start(out=bk_sb, in_=bkP[:, :])
                bias_tiles["k"] = bk_sb
            if use_bv:
                bv_sb = ws.tile([128, MPC], F32, tag="bv")
                nc.sync.dma_start(out=bv_sb, in_=bvR.ap().to_broadcast([128, MPC]))
                bias_tiles["v"] = bv_sb

            def emit_qk_proj(name, x_sb, w_sb, dst, mt_cs):
                for mt, c in mt_cs:
                    ps = ps_o.tile([128, CH], F32, tag="o", name=f"pj_{name}")
                    for kc in range(NKC):
                        nc.tensor.matmul(
                            ps,
                            lhsT=w_sb[:, kc, mt * 128 : (mt + 1) * 128],
                            rhs=x_sb[:, kc, c * CH : (c + 1) * CH],
                            start=(kc == 0),
                            stop=(kc == NKC - 1),
                        )
                    if name in bias_tiles:
                        nc.vector.tensor_scalar_add(
                            out=dst[:, mt, c * CH : (c + 1) * CH],
                            in0=ps,
                            scalar1=bias_tiles[name][:, mt : mt + 1],
                        )
                    else:
                        nc.scalar.copy(
                            out=dst[:, mt, c * CH : (c + 1) * CH], in_=ps
                        )

            def emit_v_lkt(x_sb, lkt):
                ps = ps_o.tile([128, MPC], F32, tag="o", name="pj_v")
                for kc in range(NKC):
                    nc.tensor.matmul(
                        ps,
                        lhsT=x_sb[:, kc, lkt * 128 : (lkt + 1) * 128],
                        rhs=wv_sb[:, kc, :],
                        start=(kc == 0),
                        stop=(kc == NKC - 1),
                    )
                psr = ps.rearrange("p (h d) -> p h d", h=HPC)
                if "v" in bias_tiles:
                    nc.vector.tensor_add(
                        out=Vaug[:, lkt, :, 0:64],
                        in0=psr,
                        in1=bias_tiles["v"].rearrange("p (h d) -> p h d", h=HPC),
                    )
                else:
                    nc.scalar.copy(out=Vaug[:, lkt, :, 0:64], in_=psr)

            # DMA order: x_v, x_k, x_q-half0 feed the minimal projections
            # needed to start phase B (V all-lk, K pr0, Q pr0 lq-half0); the
            # rest (x_q-half1, remaining projections) are deferred to phase-B
            # group boundaries so the gate pipeline starts ~35us earlier.
            # DMA order: x_k and x_q-half0 feed K/Q(half0) projections so the
            # gate pipeline starts ASAP; gates next; x_v follows — V-proj is
            # emitted INSIDE the first phase-B group (PV only needs it ~10
            # iterations in) so it overlaps the mul/exp pipeline.
            x_k = xs.tile([128, NKC, L], BF16, tag="x", name="x_k")
            for kc in range(NKC):
                nc.sync.dma_start(out=x_k[:, kc, :], in_=kT[:, kc, :])
            x_q = xs.tile([128, NKC, L], BF16, tag="x", name="x_q")
            for kc in range(NKC):
                nc.sync.dma_start(out=x_q[:, kc, 0:HF], in_=qT[:, kc, 0:HF])

            # both K head-pairs projected here: x_k's SBUF slot is reused by
            # x_v below, so x_k must have no readers after phase A
            emit_qk_proj(
                "k",
                x_k,
                wk_sb,
                KT,
                [(0, 0), (0, 1), (0, 2), (0, 3), (1, 0), (1, 1), (1, 2), (1, 3)],
            )
            emit_qk_proj("q", x_q, wq_sb, QT, [(0, 0), (0, 1)])

            # gate prefetch, then x_v (lk-quarters), then the x_q half
            g_tiles = {}
            NPRE = 6
            for i in range(NPRE):
                g_sb = gp.tile([128, L], gdt, tag="g")
                nc.sync.dma_start(out=g_sb, in_=gP[i])
                g_tiles[i] = g_sb
            x_v = xs.tile([128, NKC, L], BF16, tag="x", name="x_v")
            for quart in range(4):
                nc.sync.dma_start(
                    out=x_v[:, :, quart * CH : (quart + 1) * CH],
                    in_=vT[:, :, quart * CH : (quart + 1) * CH],
                )
            for kc in range(NKC):
                nc.sync.dma_start(out=x_q[:, kc, HF:L], in_=qT[:, kc, HF:L])

            # leftover projections, injected at phase-B group boundaries
            # (PSUM is free there; the PE has global slack vs the DVE wall)
            boundary_proj = {
                0: [("q", x_q, wq_sb, QT, [(0, 2), (0, 3)])],
                1: [("q", x_q, wq_sb, QT, [(1, 0), (1, 1)])],
                2: [("q", x_q, wq_sb, QT, [(1, 2), (1, 3)])],
            }

            OT = opl.tile([128, 2, L], BF16, tag="ot")

            def emit_pv(pr, half, lkt, p_sb, o_ps):
                for hp in range(2):
                    for c in range(2):
                        nc.tensor.matmul(
                            o_ps[hp][:, c * CH : (c + 1) * CH],
                            lhsT=Vaug[:, lkt, 2 * pr + hp, :],
                            rhs=p_sb[:, c * 1024 + hp * CH : c * 1024 + (hp + 1) * CH],
                            start=(lkt == 0),
                            stop=(lkt == NLKT - 1),
                        )

            gi = 0  # running gate block index (== (pr*2+half)*NLKT+lkt)
            ti = 0  # running elementwise tile index for the gpsimd split
            for pr in range(2):
                for half in range(2):
                    grp = pr * 2 + half
                    if grp > 0:
                        for args in boundary_proj.get(grp - 1, ()):
                            emit_qk_proj(*args)
                    def alloc_ops():
                        return {
                            hp: ps_o.tile(
                                [128, 1024], F32, tag="o", name=f"o_{pr}_{half}_{hp}"
                            )
                            for hp in range(2)
                        }

                    # group 0 defers V-proj (and hence o_ps + PV) into the
                    # loop so the mul/exp pipeline starts before x_v arrives
                    o_ps = None if grp == 0 else alloc_ops()
                    backlog = []
                    for lkt in range(NLKT):
                        if grp == 0 and lkt == 5:
                            for vt in range(NLKT):
                                emit_v_lkt(x_v, vt)
                            o_ps = alloc_ops()
                        if gi in g_tiles:
                            g_sb = g_tiles.pop(gi)
                        else:
                            g_sb = gp.tile([128, L], gdt, tag="g")
                            nc.sync.dma_start(out=g_sb, in_=gP[gi])
                        gi += 1
                        tmp = tp.tile([128, L], F32, tag="tmp")
                        p_sb = pp.tile([128, L], BF16, tag="p")
                        for c in range(2):
                            s_w = ps_s.tile([128, 1024], F32, tag="s", name="s_att")
                            for hp in range(2):
                                nc.tensor.matmul(
                                    s_w[:, hp * CH : (hp + 1) * CH],
                                    lhsT=KT[
                                        hp * 64 : hp * 64 + 64,
                                        pr,
                                        lkt * 128 : (lkt + 1) * 128,
                                    ],
                                    rhs=QT[
                                        hp * 64 : hp * 64 + 64,
                                        pr,
                                        half * HF + c * CH : half * HF + (c + 1) * CH,
                                    ],
                                    start=True,
                                    stop=True,
                                )
                            eng = (
                                nc.gpsimd
                                if (gp_mod and ti % gp_mod == gp_mod - 1)
                                else nc.vector
                            )
                            ti += 1
                            eng.tensor_mul(
                                out=tmp[:, c * 1024 : (c + 1) * 1024],
                                in0=s_w,
                                in1=g_sb[:, c * 1024 : (c + 1) * 1024],
                            )
                        nc.scalar.activation(
                            out=p_sb, in_=tmp, func=AF.Exp, bias=ebias, scale=1.0
                        )
                        if masked:
                            mb_sb = gp.tile([128, 1024], BF16, tag="mb")
                            nc.sync.dma_start(
                                out=mb_sb,
                                in_=mbT[
                                    lkt * 128 : (lkt + 1) * 128,
                                    half * HF : (half + 1) * HF,
                                ],
                            )
                            for c in range(2):
                                for hp in range(2):
                                    nc.vector.tensor_mul(
                                        out=p_sb[
                                            :,
                                            c * 1024 + hp * CH : c * 1024 + (hp + 1) * CH,
                                        ],
                                        in0=p_sb[
                                            :,
                                            c * 1024 + hp * CH : c * 1024 + (hp + 1) * CH,
                                        ],
                                        in1=mb_sb[:, c * CH : (c + 1) * CH],
                                    )
                        backlog.append((lkt, p_sb))
                        if o_ps is not None:
                            drained = 0
                            while (
                                backlog
                                and backlog[0][0] <= lkt - 1
                                and drained < 2
                            ):
                                it, pb = backlog.pop(0)
                                emit_pv(pr, half, it, pb, o_ps)
                                drained += 1
                    for it, pb in backlog:
                        emit_pv(pr, half, it, pb, o_ps)
                    # normalization: ACT copies both heads' denominators into
                    # one SBUF tile, one fast DVE reciprocal, two scale-muls
                    dd = rp.tile([128, 1024], F32, tag="d")
                    for hp in range(2):
                        nc.scalar.copy(
                            out=dd[hp * 64 : (hp + 1) * 64, :],
                            in_=o_ps[hp][64:128, :],
                        )
                    r_sb = rp.tile([128, 1024], F32, tag="r")
                    nc.vector.reciprocal_approx_fast(r_sb, dd)
                    for hp in range(2):
                        nc.vector.tensor_mul(
                            out=OT[
                                hp * 64 : hp * 64 + 64,
                                pr,
                                half * HF : (half + 1) * HF,
                            ],
                            in0=o_ps[hp][0:64, :],
                            in1=r_sb[hp * 64 : (hp + 1) * 64, :],
                        )
                    nc.sync.dma_start(
                        out=oT[:, pr, half * HF : (half + 1) * HF],
                        in_=OT[:, pr, half * HF : (half + 1) * HF],
                    )

    nc.finalize()
    return nc


def build_l2(use_bo: bool, use_gamma: bool, use_beta: bool):
    nc = bacc.Bacc("TRN2", target_bir_lowering=False)

    oTf = nc.declare_dram_parameter("oTf", [128, NKC, CH], BF16, isOutput=False)
    woTs = nc.declare_dram_parameter("woTs", [128, NKC, D], BF16, isOutput=False)
    qres = nc.declare_dram_parameter("qres", [4, 128, D], BF16, isOutput=False)
    if use_bo:
        boR = nc.declare_dram_parameter("boR", [1, D], F32, isOutput=False)
    if use_gamma:
        gaR = nc.declare_dram_parameter("gaR", [1, D], F32, isOutput=False)
    if use_beta:
        beR = nc.declare_dram_parameter("beR", [1, D], F32, isOutput=False)
    yout = nc.declare_dram_parameter("yout", [4, 128, D], F32, isOutput=True)

    with tile.TileContext(nc) as tc:
        with (
            tc.tile_pool(name="ins", bufs=1) as ins,
            tc.tile_pool(name="res", bufs=4) as res,
            tc.tile_pool(name="xb", bufs=4) as xb,
            tc.tile_pool(name="st", bufs=8) as st,
            tc.tile_pool(name="ps", bufs=8, space="PSUM") as psp,
        ):
            # few fat DMAs, dispatched immediately: matmul inputs first
            # (wo in n-halves so the first matmuls start sooner), then the
            # residuals in consumption order
            oT_sb = ins.tile([128, NKC, CH], BF16, tag="ot")
            nc.sync.dma_start(out=oT_sb, in_=oTf[:, :, :])
            wo_sb = ins.tile([128, NKC, D], BF16, tag="wo")
            nc.sync.dma_start(out=wo_sb[:, :, 0:512], in_=woTs[:, :, 0:512])
            q_sbs = []
            for m in range(4):
                q_sb = res.tile([128, D], BF16, tag="q", name=f"q_{m}")
                nc.sync.dma_start(out=q_sb, in_=qres[m, :, :])
                q_sbs.append(q_sb)
            nc.sync.dma_start(out=wo_sb[:, :, 512:D], in_=woTs[:, :, 512:D])
            warm = ins.tile([128, 2, 128], BF16, tag="warm")
            nc.vector.memset(warm, 1.0)
            eps_sb = ins.tile([128, 1], F32, tag="eps")
            nc.vector.memset(eps_sb, EPS)
            bo_sb = ga_sb = be_sb = None
            if use_bo:
                bo_sb = ins.tile([128, D], F32, tag="bo")
                nc.sync.dma_start(out=bo_sb, in_=boR.ap().to_broadcast([128, D]))
            if use_gamma:
                ga_sb = ins.tile([128, D], F32, tag="ga")
                nc.sync.dma_start(out=ga_sb, in_=gaR.ap().to_broadcast([128, D]))
            if use_beta:
                be_sb = ins.tile([128, D], F32, tag="be")
                nc.sync.dma_start(out=be_sb, in_=beR.ap().to_broadcast([128, D]))

            fused_ln = bo_sb is None

            # p-state / ldweights warmup while the input DMAs land
            junk = psp.tile([128, 512], F32, tag="mm", name="junk")
            for _ in range(24):
                nc.tensor.matmul(
                    junk[:, 0:128],
                    lhsT=warm[:, 0, :],
                    rhs=warm[:, 1, :],
                    start=True,
                    stop=True,
                )
            nc.scalar.copy(out=warm[0:1, 0, :], in_=junk[0:1, 0:128])

            # all 64 matmuls back-to-back so the PE streams at full p-state;
            # n=1 groups are interleaved late so they trail the wo n-half DMA
            pss = {}
            for m, n in [(0, 0), (1, 0), (0, 1), (2, 0), (1, 1), (3, 0), (2, 1), (3, 1)]:
                ps = psp.tile([128, 512], F32, tag="mm", name=f"mm_{m}_{n}")
                for kc in range(NKC):
                    nc.tensor.matmul(
                        ps,
                        lhsT=oT_sb[:, kc, m * 128 : (m + 1) * 128],
                        rhs=wo_sb[:, kc, n * 512 : (n + 1) * 512],
                        start=(kc == 0),
                        stop=(kc == NKC - 1),
                    )
                pss[(m, n)] = ps

            for m in range(4):
                q_sb = q_sbs[m]
                x = xb.tile([128, D], F32, tag="x")
                accs = st.tile([128, 2], F32, tag="accs")
                for n in range(2):
                    ps = pss[(m, n)]
                    if fused_ln:
                        # x = fc + residual, and accumulate the row-sum
                        nc.vector.scalar_tensor_tensor(
                            out=x[:, n * 512 : (n + 1) * 512],
                            in0=ps,
                            scalar=1.0,
                            in1=q_sb[:, n * 512 : (n + 1) * 512],
                            op0=AOP.mult,
                            op1=AOP.add,
                            accum_out=accs[:, n : n + 1],
                        )
                    else:
                        nc.vector.tensor_add(
                            out=x[:, n * 512 : (n + 1) * 512],
                            in0=ps,
                            in1=q_sb[:, n * 512 : (n + 1) * 512],
                        )
                if fused_ln:
                    # variance via ACT: ssq = sum(x^2) (Square writes a scratch
                    # we ignore); mean/var assembled from the two accumulators
                    scr = xb.tile([128, D], F32, tag="scr")
                    ssq = st.tile([128, 1], F32, tag="ssq")
                    nc.scalar.activation(
                        out=scr, in_=x, func=AF.Square, accum_out=ssq
                    )
                    mu = st.tile([128, 1], F32, tag="mu")
                    nc.vector.tensor_scalar(
                        out=mu,
                        in0=accs[:, 0:1],
                        scalar1=accs[:, 1:2],
                        scalar2=1.0 / D,
                        op0=AOP.add,
                        op1=AOP.mult,
                    )
                    musq = st.tile([128, 1], F32, tag="musq")
                    nc.vector.tensor_mul(out=musq, in0=mu, in1=mu)
                    var = st.tile([128, 1], F32, tag="var")
                    nc.vector.tensor_scalar(
                        out=var,
                        in0=ssq,
                        scalar1=1.0 / D,
                        scalar2=musq,
                        op0=AOP.mult,
                        op1=AOP.subtract,
                    )
                    std = st.tile([128, 1], F32, tag="std")
                    nc.scalar.activation(
                        out=std, in_=var, func=AF.Sqrt, bias=eps_sb, scale=1.0
                    )
                else:
                    if bo_sb is not None:
                        nc.vector.tensor_add(out=x, in0=x, in1=bo_sb)
                    stats = st.tile([128, 2, 6], F32, tag="stats")
                    for half in range(2):
                        nc.vector.bn_stats(
                            out=stats[:, half, :],
                            in_=x[:, half * 512 : (half + 1) * 512],
                        )
                    mv = st.tile([128, 2], F32, tag="mv")
                    nc.vector.bn_aggr(out=mv, in_=stats)
                    mu = mv[:, 0:1]
                    std = st.tile([128, 1], F32, tag="std")
                    nc.scalar.activation(
                        out=std, in_=mv[:, 1:2], func=AF.Sqrt, bias=eps_sb, scale=1.0
                    )
                rstd = st.tile([128, 1], F32, tag="rstd")
                nc.vector.reciprocal(out=rstd, in_=std)
                y = xb.tile([128, D], F32, tag="y")
                nc.vector.tensor_scalar(
                    out=y,
                    in0=x,
                    scalar1=mu,
                    scalar2=rstd,
                    op0=AOP.subtract,
                    op1=AOP.mult,
                )
                if ga_sb is not None:
                    nc.vector.tensor_mul(out=y, in0=y, in1=ga_sb)
                if be_sb is not None:
                    nc.vector.tensor_add(out=y, in0=y, in1=be_sb)
                nc.sync.dma_start(out=yout[m, :, :], in_=y)

    nc.finalize()
    return nc


_L1_CACHE = {}
_L2_CACHE = {}
LAST_RUNS = []  # (tag, nc, in_maps) of the most recent kernel() call, for profiling


def _pack_gate(g, gate_fp8):
    """[HPC, L(lq), L(lk)] -> [4*NLKT, 128, L] in consumption order.

    block i = ((pr*2+half)*NLKT + lkt); within a block, col = c*1024+hp*512+j,
    partition p = lk within the tile.
    """
    gr = g.reshape(2, 2, 2, 2, CH, NLKT, 128)  # [pr, hp, half, c, j, lkt, p]
    gt = gr.transpose(0, 2, 5, 6, 3, 1, 4)  # [pr, half, lkt, p, c, hp, j]
    gt = np.ascontiguousarray(gt).reshape(4 * NLKT, 128, L)
    return gt.astype(NPFP8 if gate_fp8 else NPBF16)


def kernel(
    q, k, v, k_gate, mask, wq, bq, wk, bk, wv, bv, wo, bo, gamma, beta
):
    q = np.asarray(q, np.float32)
    k = np.asarray(k, np.float32)
    v = np.asarray(v, np.float32)
    k_gate = np.asarray(k_gate, np.float32)
    mask = np.asarray(mask)
    wq = np.asarray(wq, np.float32)
    wk = np.asarray(wk, np.float32)
    wv = np.asarray(wv, np.float32)
    wo = np.asarray(wo, np.float32)
    bq = np.asarray(bq, np.float32)
    bk = np.asarray(bk, np.float32)
    bv = np.asarray(bv, np.float32)
    bo = np.asarray(bo, np.float32)
    gamma = np.asarray(gamma, np.float32)
    beta = np.asarray(beta, np.float32)

    masked = bool(mask.any())
    use_bq = bool(np.any(bq))
    use_bk = bool(np.any(bk))
    use_bv = bool(np.any(bv))
    use_bo = bool(np.any(bo))
    use_gamma = bool(np.any(gamma != 1.0))
    use_beta = bool(np.any(beta))

    temp = float(np.float32(np.power(DK, 0.5)))

    key1 = (masked, use_bq, use_bk, use_bv, GATE_FP8, GP_MOD)
    if key1 not in _L1_CACHE:
        _L1_CACHE[key1] = build_l1(*key1)
    nc1 = _L1_CACHE[key1]

    # ---- stage launch-1 inputs ----
    xT = {}  # (name, b) -> [128, NKC, L] bf16
    for b in range(B):
        xT[("q", b)] = _bf(_kc_layout(q[b].T))
        xT[("k", b)] = _bf(_kc_layout(k[b].T))
        xT[("v", b)] = _bf(_kc_layout(v[b].T))
    wts = {}  # (name, hg) -> [128, NKC, MPC] bf16
    for hg in range(4):
        sl = slice(hg * MPC, (hg + 1) * MPC)
        wts[("q", hg)] = _bf(_kc_layout(wq[sl].T / temp))
        wts[("k", hg)] = _bf(_kc_layout(wk[sl].T))
        wts[("v", hg)] = _bf(_kc_layout(wv[sl].T))

    in_maps = []
    for c in range(NCORE):
        b, hg = c // 4, c % 4
        hsl = slice(hg * HPC, (hg + 1) * HPC)
        m = {
            "qT": xT[("q", b)],
            "kT": xT[("k", b)],
            "vT": xT[("v", b)],
            "wqT": wts[("q", hg)],
            "wkT": wts[("k", hg)],
            "wvT": wts[("v", hg)],
            "gP": _pack_gate(k_gate[b, hsl], GATE_FP8),
        }
        if use_bq:
            m["bqP"] = np.ascontiguousarray(
                (bq[hg * MPC : (hg + 1) * MPC] / temp).reshape(2, 128).T
            )
        if use_bk:
            m["bkP"] = np.ascontiguousarray(
                bk[hg * MPC : (hg + 1) * MPC].reshape(2, 128).T
            )
        if use_bv:
            m["bvR"] = bv[hg * MPC : (hg + 1) * MPC].reshape(1, MPC).copy()
        if masked:
            m["mbT"] = _bf((~mask[b]).astype(np.float32).T)
        in_maps.append(m)

    LAST_RUNS.clear()
    LAST_RUNS.append(("L1", nc1, in_maps))
    res1 = run_bass_kernel_spmd(nc1, in_maps, list(range(NCORE)))

    # assemble O^T per batch: [H*DV, L] bf16
    OTb = []
    for b in range(B):
        parts = []
        for hg in range(4):
            r = res1.results[b * 4 + hg]["oT"]  # [128, 2, L] bf16
            parts.append(np.ascontiguousarray(r.transpose(1, 0, 2)).reshape(MPC, L))
        OTb.append(np.concatenate(parts, axis=0))  # [1024, L]

    key2 = (use_bo, use_gamma, use_beta)
    if key2 not in _L2_CACHE:
        _L2_CACHE[key2] = build_l2(*key2)
    nc2 = _L2_CACHE[key2]

    woTs = _bf(_kc_layout(wo.T))
    in_maps2 = []
    for c in range(NCORE):
        b, rchunk = c // 4, c % 4
        rows = slice(rchunk * CH, (rchunk + 1) * CH)
        otf = OTb[b][:, rows]  # [1024, 512] bf16
        m = {
            "oTf": np.ascontiguousarray(
                otf.reshape(NKC, 128, CH).transpose(1, 0, 2)
            ),
            "woTs": woTs,
            "qres": _bf(q[b, rows].reshape(4, 128, D)),
        }
        if use_bo:
            m["boR"] = bo.reshape(1, D).copy()
        if use_gamma:
            m["gaR"] = gamma.reshape(1, D).copy()
        if use_beta:
            m["beR"] = beta.reshape(1, D).copy()
        in_maps2.append(m)

    LAST_RUNS.append(("L2", nc2, in_maps2))
    res2 = run_bass_kernel_spmd(nc2, in_maps2, list(range(NCORE)))

    out = np.empty((B, L, D), np.float32)
    for c in range(NCORE):
        b, rchunk = c // 4, c % 4
        out[b, rchunk * CH : (rchunk + 1) * CH] = res2.results[c]["yout"].reshape(
            CH, D
        )
    return out
